# revision 10
# baseline (speedup 1.0000x reference)
"""v7: v6 render path + geometric empty-space fast path.

Volume-rendering early termination, done rigorously on the host from
cam_pose alone: the exclusive cumprod means sample s only contributes
through the product of earlier occupancies.  If a ray's FIRST sample has
trilinear weight <= EPS on real (non-pad) voxels, then occ[0] <= EPS + 1e-12,
so every later sample's transmittance is <= EPS + 1e-12 and the ray's total
class logit is bounded by 128*EPS — far below the 2e-2 gate for EPS=1e-6.
When every ray is provably trivial the exact output is [1, 0, ..., 0] per
pixel (to <= 1.3e-4), written by a tiny constant kernel.  Otherwise the full
v6 render path runs (correct for arbitrary inputs).

v6 render path: per 4-tile group the coordinate/index math runs as [128, 512]
ops (4x less instr overhead); gathers/lerps/cumprod stay per 128-ray tile.
Table and gathered corner data are fp16 (lerp math accumulates in fp32).
"""

import numpy as np

import concourse.bacc as bacc
import concourse.bass as bass
import concourse.mybir as mybir
from concourse.tile import TileContext
from concourse.bass_utils import run_bass_kernel_spmd

F32 = mybir.dt.float32
F16 = mybir.dt.float16
I32 = mybir.dt.int32

B = 2
VOX = 64
C = 13
H = W = 128
S = 128
NEAR, FAR = 0.9, 2.2
CAM_FOV = 0.8

N_CORES = 8
STRIPS = 4
ROWS_PER_CORE = H // STRIPS
RAYS_PER_CORE = ROWS_PER_CORE * W
NT = RAYS_PER_CORE // 128       # 32 ray tiles
GRP = 4                         # tiles fused per group for small ops
NG = NT // GRP                  # 8 groups
SF = S * GRP                    # 512 fused free dim

DP = VOX + 2
XSTR = C
YSTR = DP * C
ZSTR = DP * DP * C
TABLE = DP * DP * DP * C
CLIP_HI = float(np.float32((DP - 1) - 1e-4))

# expanded-table geometry (render path): one row per cell (z,y,x), row =
# 4 corner blocks (dz,dy) x 13 channels = 52 fp16; a sample gathers rows
# x0 and x0+1 in one 104-element contiguous fetch.
CROW = 4 * C                      # 52
NCELL_ZY = DP * DP                # 4356 (z,y) rows
ZYCHUNKS = 35                     # ceil-ish: 35*128 = 4480 >= 4356 + shifts
VP_ROWS = ZYCHUNKS * 128 + 68     # vp (z,y)-row view incl. build overread pad
E4_ROWS = ZYCHUNKS * 128 * DP     # e4 cells incl. build pad
# floor(x) for x>=0 as round-nearest(x - FLOOR_EPS); cast-to-int is RNE.
FLOOR_EPS = 0.4999999

AL = mybir.AluOpType
ACTF = mybir.ActivationFunctionType


def _build_program():
    nc = bacc.Bacc("TRN2", target_bir_lowering=False, debug=False)

    vox_in = nc.dram_tensor("vox", [VOX * VOX, VOX * C], F32, kind="ExternalInput")
    # raya[r, (grp*3+k)*4 + jj] = a[(grp*4+jj)*128 + r, k]
    raya_in = nc.dram_tensor("raya", [128, NT * 3], F32, kind="ExternalInput")
    cvec_in = nc.dram_tensor("cvec", [128, 3], F32, kind="ExternalInput")
    trep_in = nc.dram_tensor("trep", [128, S], F32, kind="ExternalInput")
    out_dram = nc.dram_tensor("out", [RAYS_PER_CORE, C], F32, kind="ExternalOutput")

    vp = nc.dram_tensor("vp", [VP_ROWS * DP * C, 1], F16, kind="Internal")
    e4 = nc.dram_tensor("e4", [E4_ROWS, CROW], F16, kind="Internal")

    with TileContext(nc) as tc:
        with (
            tc.tile_pool(name="const", bufs=1) as cpool,
            tc.tile_pool(name="prep", bufs=2) as ppool,
            tc.tile_pool(name="build", bufs=2) as bpool,
            tc.tile_pool(name="grp", bufs=1) as wpool,
            tc.tile_pool(name="small", bufs=2) as spool,
            tc.tile_pool(name="gath", bufs=2) as gpool,
            tc.tile_pool(name="lerp", bufs=1) as lpool,
        ):
            # ---- constants ----
            trep_t = cpool.tile([128, S], F32, tag="trep")
            nc.sync.dma_start(trep_t[:], trep_in[:])
            raya_t = cpool.tile([128, NT * 3], F32, tag="raya")
            nc.sync.dma_start(raya_t[:], raya_in[:])
            cvec_t = cpool.tile([128, 3], F32, tag="cvec")
            nc.sync.dma_start(cvec_t[:], cvec_in[:])
            zeros_t = cpool.tile([128, S], F32, tag="zeros")
            nc.vector.memset(zeros_t[:], 0.0)
            zface_t = cpool.tile([DP, YSTR], F16, tag="zface")
            nc.vector.memset(zface_t[:], 0.0)

            # ---- zero pad faces of vp ----
            for z in (0, DP - 1):
                dst = vp[z * ZSTR : (z + 1) * ZSTR, :].rearrange(
                    "(y x) o -> y (x o)", x=YSTR
                )
                nc.sync.dma_start(dst, zface_t[:, :])
            mid = vp[ZSTR : (DP - 1) * ZSTR, :].rearrange(
                "(z w) o -> z (w o)", w=ZSTR
            )
            nc.sync.dma_start(mid[:, 0:YSTR], zface_t[0:VOX, :])
            nc.sync.dma_start(mid[:, (DP - 1) * YSTR : DP * YSTR], zface_t[0:VOX, :])
            mid3 = vp[ZSTR : (DP - 1) * ZSTR, :].rearrange(
                "(z y w) o -> z y (w o)", y=DP, w=YSTR
            )
            nc.sync.dma_start(mid3[:, 1 : DP - 1, 0:XSTR], zface_t[0:VOX, 0 : VOX * XSTR])
            nc.sync.dma_start(
                mid3[:, 1 : DP - 1, (DP - 1) * XSTR : DP * XSTR],
                zface_t[0:VOX, 0 : VOX * XSTR],
            )

            # ---- sigmoid(vox) -> fp16 interior of vp ----
            for z in range(VOX):
                slab_t = ppool.tile([128, VOX * VOX * C // 128], F32, tag="slab")
                nc.sync.dma_start(slab_t[:], vox_in[z * VOX : (z + 1) * VOX, :])
                sig_t = ppool.tile([128, VOX * VOX * C // 128], F16, tag="sig")
                nc.scalar.activation(sig_t[:], slab_t[:], ACTF.Sigmoid)
                dst = vp[
                    (z + 1) * ZSTR + YSTR : (z + 1) * ZSTR + (DP - 1) * YSTR, :
                ].rearrange("(y w) o -> y (w o)", w=YSTR)
                nc.sync.dma_start(dst[:, XSTR : (DP - 1) * XSTR], sig_t[:])

            # ---- build expanded table e4[cell] = 4 (dz,dy) corner x-lines ----
            vp_rows = vp[0 : VP_ROWS * DP * C, :].rearrange(
                "(r w) o -> r (w o)", w=DP * C
            )
            for ck in range(ZYCHUNKS):
                base = ck * 128
                ob = bpool.tile([128, DP * CROW], F16, tag="obuild")
                o3 = ob[:].rearrange("p (x b c) -> p x b c", b=4, c=C)
                for dz in (0, 1):
                    for dy in (0, 1):
                        st = bpool.tile([128, DP * C], F16, tag=f"src{dz}{dy}")
                        nc.sync.dma_start(
                            st[:],
                            vp_rows[base + dz * DP + dy : base + dz * DP + dy + 128, :],
                        )
                        s3 = st[:].rearrange("p (x c) -> p x c", c=C)
                        if dy:
                            nc.vector.tensor_copy(out=o3[:, :, dz * 2 + dy, :], in_=s3)
                        else:
                            nc.scalar.activation(
                                o3[:, :, dz * 2 + dy, :], s3, ACTF.Identity
                            )
                nc.sync.dma_start(
                    e4[base * DP : (base + 128) * DP, :].rearrange(
                        "(p x) w -> p (x w)", x=DP
                    ),
                    ob[:],
                )

            # ---- main loop: 8 groups of 4 ray tiles ----
            for gi in range(NG):
                cx_ap = [cvec_t[:, k : k + 1] for k in range(3)]

                # fused [128, 512] coordinate pipeline
                i0f = []
                frac = []
                for k in range(3):
                    q4 = wpool.tile([128, SF], F32, tag=f"q{k}")
                    for jj in range(GRP):
                        col = (gi * 3 + k) * GRP + jj
                        nc.scalar.activation(
                            q4[:, jj * S : (jj + 1) * S], trep_t[:], ACTF.Identity,
                            bias=cx_ap[k], scale=raya_t[:, col : col + 1],
                        )
                    cp = wpool.tile([128, SF], F32, tag=f"cp{k}")
                    nc.vector.tensor_scalar(
                        out=cp[:], in0=q4[:], scalar1=0.0, scalar2=CLIP_HI,
                        op0=AL.max, op1=AL.min,
                    )
                    # floor: int cast is round-nearest-even, so shift by just
                    # under one half first (cp >= 0 post-clip)
                    ii = wpool.tile([128, SF], I32, tag=f"ii{k}")
                    nc.vector.tensor_scalar(
                        out=ii[:], in0=cp[:], scalar1=-FLOOR_EPS, scalar2=None,
                        op0=AL.add,
                    )
                    fi = wpool.tile([128, SF], F32, tag=f"fi{k}")
                    nc.vector.tensor_copy(out=fi[:], in_=ii[:])
                    fr = wpool.tile([128, SF], F32, tag=f"fr{k}")
                    nc.vector.tensor_tensor(
                        out=fr[:], in0=cp[:], in1=fi[:], op=AL.subtract
                    )
                    i0f.append(fi)
                    frac.append(fr)

                # cell index = (i0z*DP + i0y)*DP + i0x  (exact in f32, < 2^24)
                m2 = wpool.tile([128, SF], F32, tag="m2")
                nc.vector.scalar_tensor_tensor(
                    out=m2[:], in0=i0f[1][:], scalar=float(DP), in1=i0f[2][:],
                    op0=AL.mult, op1=AL.add,
                )
                idxt = wpool.tile([128, SF], I32, tag="idxt")
                nc.vector.scalar_tensor_tensor(
                    out=idxt[:], in0=i0f[0][:], scalar=float(DP * DP), in1=m2[:],
                    op0=AL.mult, op1=AL.add,
                )

                # per-tile gathers + lerp + render
                for jj in range(GRP):
                    ssl = slice(jj * S, (jj + 1) * S)
                    j = gi * GRP + jj
                    G = gpool.tile([128, S * 2 * CROW], F16, tag="G")
                    for s in range(S):
                        col = jj * S + s
                        nc.gpsimd.indirect_dma_start(
                            out=G[:, s * 2 * CROW : (s + 1) * 2 * CROW],
                            out_offset=None,
                            in_=e4[:],
                            in_offset=bass.IndirectOffsetOnAxis(
                                ap=idxt[:, col : col + 1], axis=0
                            ),
                        )

                    # frac broadcasts (all on scalar: gpsimd is gather-bound)
                    fx52 = lpool.tile([128, S * 52], F16, tag="fx52")
                    nc.scalar.activation(
                        fx52[:].rearrange("p (s w) -> p s w", w=52),
                        frac[2][:, ssl].unsqueeze(-1).broadcast_to([128, S, 52]),
                        ACTF.Identity,
                    )
                    fy26 = lpool.tile([128, S * 26], F16, tag="fy26")
                    nc.scalar.activation(
                        fy26[:].rearrange("p (s w) -> p s w", w=26),
                        frac[1][:, ssl].unsqueeze(-1).broadcast_to([128, S, 26]),
                        ACTF.Identity,
                    )
                    fz13 = lpool.tile([128, S * C], F16, tag="fz13")
                    nc.scalar.activation(
                        fz13[:].rearrange("p (s w) -> p s w", w=C),
                        frac[0][:, ssl].unsqueeze(-1).broadcast_to([128, S, C]),
                        ACTF.Identity,
                    )

                    # x-lerp: [p, s, two, 52] -> vx [p, s, 52]
                    G4 = G[:].rearrange("p (s two w) -> p s two w", two=2, w=CROW)
                    vx = lpool.tile([128, S * 52], F16, tag="vx")
                    vx3 = vx[:].rearrange("p (s w) -> p s w", w=52)
                    nc.vector.tensor_tensor(
                        out=vx3, in0=G4[:, :, 1, :], in1=G4[:, :, 0, :],
                        op=AL.subtract,
                    )
                    nc.vector.tensor_tensor(out=vx[:], in0=vx[:], in1=fx52[:], op=AL.mult)
                    nc.vector.tensor_tensor(
                        out=vx3, in0=vx3, in1=G4[:, :, 0, :], op=AL.add
                    )

                    # y-lerp: vx [p, s, dz, dy, 13] -> vy [p, s, dz*13]
                    vx5 = vx[:].rearrange(
                        "p (s dz dy c) -> p s dz dy c", dz=2, dy=2, c=C
                    )
                    vy = lpool.tile([128, S * 26], F16, tag="vy")
                    vy3 = vy[:].rearrange("p (s w) -> p s w", w=26)
                    vy4 = vy[:].rearrange("p (s dz c) -> p s dz c", dz=2, c=C)
                    nc.vector.tensor_tensor(
                        out=vy4, in0=vx5[:, :, :, 1, :], in1=vx5[:, :, :, 0, :],
                        op=AL.subtract,
                    )
                    nc.vector.tensor_tensor(out=vy[:], in0=vy[:], in1=fy26[:], op=AL.mult)
                    nc.vector.tensor_tensor(
                        out=vy4, in0=vy4, in1=vx5[:, :, :, 0, :], op=AL.add
                    )

                    # z-lerp -> rv [p, s, 13]
                    vy4z = vy[:].rearrange("p (s dz c) -> p s dz c", dz=2, c=C)
                    vz = lpool.tile([128, S * C], F16, tag="vz")
                    rv = vz[:].rearrange("p (s c) -> p s c", c=C)
                    nc.vector.tensor_tensor(
                        out=rv, in0=vy4z[:, :, 1, :], in1=vy4z[:, :, 0, :],
                        op=AL.subtract,
                    )
                    nc.vector.tensor_tensor(out=vz[:], in0=vz[:], in1=fz13[:], op=AL.mult)
                    nc.vector.tensor_tensor(
                        out=rv, in0=rv, in1=vy4z[:, :, 0, :], op=AL.add
                    )

                    occx = spool.tile([128, S], F32, tag="occx")
                    nc.vector.memset(occx[:, 0:1], 1.0)
                    nc.vector.tensor_scalar(
                        out=occx[:, 1:S], in0=rv[:, 0 : S - 1, 0], scalar1=1e-12,
                        scalar2=None, op0=AL.add,
                    )
                    cum = spool.tile([128, S], F32, tag="cum")
                    nc.vector.tensor_tensor_scan(
                        out=cum[:], data0=occx[:], data1=zeros_t[:], initial=1.0,
                        op0=AL.mult, op1=AL.add,
                    )

                    pot = spool.tile([128, (C - 1) * S], F32, tag="pot")
                    nc.vector.tensor_tensor(
                        out=pot[:].rearrange("p (c s) -> p s c", s=S),
                        in0=rv[:, :, 1:C],
                        in1=cum[:].unsqueeze(-1).broadcast_to([128, S, C - 1]),
                        op=AL.mult,
                    )
                    outt = spool.tile([128, C], F32, tag="outt")
                    nc.vector.tensor_reduce(
                        out=outt[:, 1:C],
                        in_=pot[:].rearrange("p (c s) -> p c s", s=S),
                        axis=mybir.AxisListType.X, op=AL.add,
                    )
                    ssum = spool.tile([128, 1], F32, tag="ssum")
                    nc.vector.tensor_reduce(
                        out=ssum[:], in_=outt[:, 1:C], axis=mybir.AxisListType.X,
                        op=AL.add,
                    )
                    nc.vector.tensor_scalar(
                        out=outt[:, 0:1], in0=ssum[:], scalar1=-1.0, scalar2=1.0,
                        op0=AL.mult, op1=AL.add,
                    )
                    nc.sync.dma_start(out_dram[j * 128 : (j + 1) * 128, :], outt[:])

    nc.compile()
    return nc


def _build_trivial_program():
    nc = bacc.Bacc("TRN2", target_bir_lowering=False, debug=False)
    out_dram = nc.dram_tensor("out", [RAYS_PER_CORE, C], F32, kind="ExternalOutput")
    with TileContext(nc) as tc:
        with tc.tile_pool(name="p", bufs=1) as pool:
            t = pool.tile([128, NT * C], F32, tag="o")
            t3 = t[:].rearrange("p (t c) -> p t c", c=C)
            nc.vector.memset(t3[:, :, 1:C], 0.0)
            nc.vector.memset(t3[:, :, 0:1], 1.0)
            dst = out_dram[:].rearrange("(p t) c -> p t c", p=128)
            half = NT // 2
            # split the output store across both HWDGE queues
            nc.sync.dma_start(dst[:, 0:half, :], t3[:, 0:half, :])
            nc.scalar.dma_start(dst[:, half:NT, :], t3[:, half:NT, :])
    nc.compile()
    return nc


TRIVIAL_EPS = 1e-6


def _all_rays_trivial(cam_pose):
    # Trilinear weight of each ray's sample 0 on real (non-pad) voxels.
    focal = H / (2.0 * np.tan(CAM_FOV / 2.0))
    v = (np.arange(H, dtype=np.float64) + 0.5 - H / 2.0) / focal
    u = (np.arange(W, dtype=np.float64) + 0.5 - W / 2.0) / focal
    dirs = np.stack(
        [np.broadcast_to(u[None, :], (H, W)),
         np.broadcast_to(v[:, None], (H, W)),
         np.ones((H, W))], axis=-1)
    wmax = 0.0
    for b in range(B):
        R = cam_pose[b, :3, :3].astype(np.float64)
        tr = cam_pose[b, :3, 3].astype(np.float64)
        if not np.all(np.isfinite(R)) or not np.all(np.isfinite(tr)):
            return False
        pts = dirs @ R.T * NEAR + tr                 # world xyz at t=NEAR
        coords = (pts[..., ::-1] + 0.5) * VOX - 0.5  # (z,y,x) index coords
        cp = np.clip(coords + 1.0, 0.0, (DP - 1) - 1e-4)
        i0 = np.floor(cp)
        f = cp - i0
        w = np.where(i0 < 1.0, f, np.where(i0 > VOX - 1, 1.0 - f, 1.0))
        wmax = max(wmax, float(w.prod(-1).max()))
    return wmax <= TRIVIAL_EPS


_NC_CACHE = {}


def _get_program():
    if "nc" not in _NC_CACHE:
        _NC_CACHE["nc"] = _build_program()
    return _NC_CACHE["nc"]


def _get_trivial_program():
    if "triv" not in _NC_CACHE:
        _NC_CACHE["triv"] = _build_trivial_program()
    return _NC_CACHE["triv"]


def _host_prep(vox, cam_pose):
    focal = H / (2.0 * np.tan(CAM_FOV / 2.0))
    v = (np.arange(H, dtype=np.float64) + 0.5 - H / 2.0) / focal
    u = (np.arange(W, dtype=np.float64) + 0.5 - W / 2.0) / focal
    dirs = np.stack(
        [np.broadcast_to(u[None, :], (H, W)),
         np.broadcast_to(v[:, None], (H, W)),
         np.ones((H, W))], axis=-1)

    t = np.linspace(NEAR, FAR, S).astype(np.float32)
    trep = np.broadcast_to(t[None, :], (128, S)).astype(np.float32).copy()

    in_maps = []
    for core in range(N_CORES):
        b = core // STRIPS
        strip = core % STRIPS
        R = cam_pose[b, :3, :3].astype(np.float64)
        tr = cam_pose[b, :3, 3].astype(np.float64)
        rd = dirs @ R.T
        a_zyx = rd[..., ::-1] * VOX
        c_zyx = tr[::-1] * VOX + (0.5 * VOX - 0.5)
        rows = slice(strip * ROWS_PER_CORE, (strip + 1) * ROWS_PER_CORE)
        a_core = a_zyx[rows].reshape(RAYS_PER_CORE, 3)   # [(j*128+r), k]
        # raya[r, (gi*3+k)*GRP + jj] = a_core[(gi*GRP+jj)*128 + r, k]
        a5 = a_core.reshape(NG, GRP, 128, 3)             # [gi, jj, r, k]
        raya = np.ascontiguousarray(
            a5.transpose(2, 0, 3, 1).reshape(128, NT * 3)
        ).astype(np.float32)
        cvec = np.broadcast_to(
            (c_zyx + 1.0).astype(np.float32)[None, :], (128, 3)
        ).copy()
        voxb = np.ascontiguousarray(
            vox[b].reshape(VOX * VOX, VOX * C)
        ).astype(np.float32)
        in_maps.append({"vox": voxb, "raya": raya, "cvec": cvec, "trep": trep})
    return in_maps


LAST_RESULTS = {}


def _install_ntff_hook():
    import sys
    import types

    if "antenv.axon_hooks" in sys.modules:
        return
    hook = None
    try:
        from trn_agent_boot.trn_boot import _ntff_profile_via_ctypes

        hook = _ntff_profile_via_ctypes("/opt/axon/libaxon_pjrt.so")
    except Exception:
        hook = None
    mod = types.ModuleType("antenv.axon_hooks")
    mod._hook = hook
    mod.get_axon_ntff_profile_hook = lambda: mod._hook
    mod.set_axon_ntff_profile_hook = lambda h: setattr(mod, "_hook", h)
    sys.modules["antenv.axon_hooks"] = mod


def _run(nc, in_maps):
    import os

    trace = bool(int(os.environ.get("BASS_KERNEL_TRACE", "0")))
    if trace:
        _install_ntff_hook()
        try:
            return run_bass_kernel_spmd(
                nc, in_maps, core_ids=list(range(N_CORES)), trace=True
            )
        except Exception as e:
            print(f"traced run failed ({type(e).__name__}: {e}); retrying untraced")
    return run_bass_kernel_spmd(nc, in_maps, core_ids=list(range(N_CORES)))


def kernel(vox, cam_pose):
    vox = np.asarray(vox)
    cam_pose = np.asarray(cam_pose)
    if _all_rays_trivial(cam_pose):
        res = _run(_get_trivial_program(), [{} for _ in range(N_CORES)])
    else:
        res = _run(_get_program(), _host_prep(vox, cam_pose))
    LAST_RESULTS["res"] = res
    out = np.empty((B, H, W, C), np.float32)
    for core in range(N_CORES):
        b = core // STRIPS
        strip = core % STRIPS
        o = res.results[core]["out"].reshape(ROWS_PER_CORE, W, C)
        out[b, strip * ROWS_PER_CORE : (strip + 1) * ROWS_PER_CORE] = o
    return out



# revision 11
# speedup vs baseline: 1.0388x; 1.0388x over previous
"""v7: v6 render path + geometric empty-space fast path.

Volume-rendering early termination, done rigorously on the host from
cam_pose alone: the exclusive cumprod means sample s only contributes
through the product of earlier occupancies.  If a ray's FIRST sample has
trilinear weight <= EPS on real (non-pad) voxels, then occ[0] <= EPS + 1e-12,
so every later sample's transmittance is <= EPS + 1e-12 and the ray's total
class logit is bounded by 128*EPS — far below the 2e-2 gate for EPS=1e-6.
When every ray is provably trivial the exact output is [1, 0, ..., 0] per
pixel (to <= 1.3e-4), written by a tiny constant kernel.  Otherwise the full
v6 render path runs (correct for arbitrary inputs).

v6 render path: per 4-tile group the coordinate/index math runs as [128, 512]
ops (4x less instr overhead); gathers/lerps/cumprod stay per 128-ray tile.
Table and gathered corner data are fp16 (lerp math accumulates in fp32).
"""

import numpy as np

import concourse.bacc as bacc
import concourse.bass as bass
import concourse.mybir as mybir
from concourse.tile import TileContext
from concourse.bass_utils import run_bass_kernel_spmd

F32 = mybir.dt.float32
F16 = mybir.dt.float16
I32 = mybir.dt.int32

B = 2
VOX = 64
C = 13
H = W = 128
S = 128
NEAR, FAR = 0.9, 2.2
CAM_FOV = 0.8

N_CORES = 8
STRIPS = 4
ROWS_PER_CORE = H // STRIPS
RAYS_PER_CORE = ROWS_PER_CORE * W
NT = RAYS_PER_CORE // 128       # 32 ray tiles
GRP = 4                         # tiles fused per group for small ops
NG = NT // GRP                  # 8 groups
SF = S * GRP                    # 512 fused free dim

DP = VOX + 2
XSTR = C
YSTR = DP * C
ZSTR = DP * DP * C
TABLE = DP * DP * DP * C
CLIP_HI = float(np.float32((DP - 1) - 1e-4))

# expanded-table geometry (render path): one row per cell (z,y,x), row =
# 4 corner blocks (dz,dy) x 13 channels = 52 fp16; a sample gathers rows
# x0 and x0+1 in one 104-element contiguous fetch.
CROW = 4 * C                      # 52
NCELL_ZY = DP * DP                # 4356 (z,y) rows
ZYCHUNKS = 35                     # ceil-ish: 35*128 = 4480 >= 4356 + shifts
VP_ROWS = ZYCHUNKS * 128 + 68     # vp (z,y)-row view incl. build overread pad
E4_ROWS = ZYCHUNKS * 128 * DP     # e4 cells incl. build pad
# floor(x) for x>=0 as round-nearest(x - FLOOR_EPS); cast-to-int is RNE.
FLOOR_EPS = 0.4999999

AL = mybir.AluOpType
ACTF = mybir.ActivationFunctionType


def _build_program():
    nc = bacc.Bacc("TRN2", target_bir_lowering=False, debug=False)

    vox_in = nc.dram_tensor("vox", [VOX * VOX, VOX * C], F32, kind="ExternalInput")
    # raya[r, (grp*3+k)*4 + jj] = a[(grp*4+jj)*128 + r, k]
    raya_in = nc.dram_tensor("raya", [128, NT * 3], F32, kind="ExternalInput")
    cvec_in = nc.dram_tensor("cvec", [128, 3], F32, kind="ExternalInput")
    trep_in = nc.dram_tensor("trep", [128, S], F32, kind="ExternalInput")
    out_dram = nc.dram_tensor("out", [RAYS_PER_CORE, C], F32, kind="ExternalOutput")

    vp = nc.dram_tensor("vp", [VP_ROWS * DP * C, 1], F16, kind="Internal")
    e4 = nc.dram_tensor("e4", [E4_ROWS, CROW], F16, kind="Internal")

    with TileContext(nc) as tc:
        with (
            tc.tile_pool(name="const", bufs=1) as cpool,
            tc.tile_pool(name="prep", bufs=2) as ppool,
            tc.tile_pool(name="build", bufs=2) as bpool,
            tc.tile_pool(name="grp", bufs=1) as wpool,
            tc.tile_pool(name="small", bufs=2) as spool,
            tc.tile_pool(name="gath", bufs=2) as gpool,
            tc.tile_pool(name="lerp", bufs=1) as lpool,
        ):
            # ---- constants ----
            trep_t = cpool.tile([128, S], F32, tag="trep")
            nc.sync.dma_start(trep_t[:], trep_in[:])
            raya_t = cpool.tile([128, NT * 3], F32, tag="raya")
            nc.sync.dma_start(raya_t[:], raya_in[:])
            cvec_t = cpool.tile([128, 3], F32, tag="cvec")
            nc.sync.dma_start(cvec_t[:], cvec_in[:])
            zeros_t = cpool.tile([128, S], F32, tag="zeros")
            nc.vector.memset(zeros_t[:], 0.0)
            zface_t = cpool.tile([DP, YSTR], F16, tag="zface")
            nc.vector.memset(zface_t[:], 0.0)

            # ---- zero pad faces of vp ----
            for z in (0, DP - 1):
                dst = vp[z * ZSTR : (z + 1) * ZSTR, :].rearrange(
                    "(y x) o -> y (x o)", x=YSTR
                )
                nc.sync.dma_start(dst, zface_t[:, :])
            mid = vp[ZSTR : (DP - 1) * ZSTR, :].rearrange(
                "(z w) o -> z (w o)", w=ZSTR
            )
            nc.sync.dma_start(mid[:, 0:YSTR], zface_t[0:VOX, :])
            nc.sync.dma_start(mid[:, (DP - 1) * YSTR : DP * YSTR], zface_t[0:VOX, :])
            mid3 = vp[ZSTR : (DP - 1) * ZSTR, :].rearrange(
                "(z y w) o -> z y (w o)", y=DP, w=YSTR
            )
            nc.sync.dma_start(mid3[:, 1 : DP - 1, 0:XSTR], zface_t[0:VOX, 0 : VOX * XSTR])
            nc.sync.dma_start(
                mid3[:, 1 : DP - 1, (DP - 1) * XSTR : DP * XSTR],
                zface_t[0:VOX, 0 : VOX * XSTR],
            )

            # ---- sigmoid(vox) -> fp16 interior of vp ----
            for z in range(VOX):
                slab_t = ppool.tile([128, VOX * VOX * C // 128], F32, tag="slab")
                nc.sync.dma_start(slab_t[:], vox_in[z * VOX : (z + 1) * VOX, :])
                sig_t = ppool.tile([128, VOX * VOX * C // 128], F16, tag="sig")
                nc.scalar.activation(sig_t[:], slab_t[:], ACTF.Sigmoid)
                dst = vp[
                    (z + 1) * ZSTR + YSTR : (z + 1) * ZSTR + (DP - 1) * YSTR, :
                ].rearrange("(y w) o -> y (w o)", w=YSTR)
                nc.sync.dma_start(dst[:, XSTR : (DP - 1) * XSTR], sig_t[:])

            # ---- build expanded table e4[cell] = 4 (dz,dy) corner x-lines ----
            vp_rows = vp[0 : VP_ROWS * DP * C, :].rearrange(
                "(r w) o -> r (w o)", w=DP * C
            )
            for ck in range(ZYCHUNKS):
                base = ck * 128
                ob = bpool.tile([128, DP * CROW], F16, tag="obuild")
                o3 = ob[:].rearrange("p (x b c) -> p x b c", b=4, c=C)
                for dz in (0, 1):
                    for dy in (0, 1):
                        st = bpool.tile([128, DP * C], F16, tag=f"src{dz}{dy}")
                        nc.sync.dma_start(
                            st[:],
                            vp_rows[base + dz * DP + dy : base + dz * DP + dy + 128, :],
                        )
                        s3 = st[:].rearrange("p (x c) -> p x c", c=C)
                        if dy:
                            nc.vector.tensor_copy(out=o3[:, :, dz * 2 + dy, :], in_=s3)
                        else:
                            nc.scalar.activation(
                                o3[:, :, dz * 2 + dy, :], s3, ACTF.Identity
                            )
                nc.sync.dma_start(
                    e4[base * DP : (base + 128) * DP, :].rearrange(
                        "(p x) w -> p (x w)", x=DP
                    ),
                    ob[:],
                )

            # ---- main loop: 8 groups of 4 ray tiles ----
            for gi in range(NG):
                cx_ap = [cvec_t[:, k : k + 1] for k in range(3)]

                # fused [128, 512] coordinate pipeline
                i0f = []
                frac = []
                for k in range(3):
                    q4 = wpool.tile([128, SF], F32, tag=f"q{k}")
                    for jj in range(GRP):
                        col = (gi * 3 + k) * GRP + jj
                        nc.scalar.activation(
                            q4[:, jj * S : (jj + 1) * S], trep_t[:], ACTF.Identity,
                            bias=cx_ap[k], scale=raya_t[:, col : col + 1],
                        )
                    cp = wpool.tile([128, SF], F32, tag=f"cp{k}")
                    nc.vector.tensor_scalar(
                        out=cp[:], in0=q4[:], scalar1=0.0, scalar2=CLIP_HI,
                        op0=AL.max, op1=AL.min,
                    )
                    # floor: int cast is round-nearest-even, so shift by just
                    # under one half first (cp >= 0 post-clip)
                    ii = wpool.tile([128, SF], I32, tag=f"ii{k}")
                    nc.vector.tensor_scalar(
                        out=ii[:], in0=cp[:], scalar1=-FLOOR_EPS, scalar2=None,
                        op0=AL.add,
                    )
                    fi = wpool.tile([128, SF], F32, tag=f"fi{k}")
                    nc.vector.tensor_copy(out=fi[:], in_=ii[:])
                    fr = wpool.tile([128, SF], F32, tag=f"fr{k}")
                    nc.vector.tensor_tensor(
                        out=fr[:], in0=cp[:], in1=fi[:], op=AL.subtract
                    )
                    i0f.append(fi)
                    frac.append(fr)

                # cell index = (i0z*DP + i0y)*DP + i0x  (exact in f32, < 2^24)
                m2 = wpool.tile([128, SF], F32, tag="m2")
                nc.vector.scalar_tensor_tensor(
                    out=m2[:], in0=i0f[1][:], scalar=float(DP), in1=i0f[2][:],
                    op0=AL.mult, op1=AL.add,
                )
                idxt = wpool.tile([128, SF], I32, tag="idxt")
                nc.vector.scalar_tensor_tensor(
                    out=idxt[:], in0=i0f[0][:], scalar=float(DP * DP), in1=m2[:],
                    op0=AL.mult, op1=AL.add,
                )

                # per-tile gathers + lerp + render
                for jj in range(GRP):
                    ssl = slice(jj * S, (jj + 1) * S)
                    j = gi * GRP + jj
                    G = gpool.tile([128, S * 2 * CROW], F16, tag="G")
                    for s in range(S):
                        col = jj * S + s
                        nc.gpsimd.indirect_dma_start(
                            out=G[:, s * 2 * CROW : (s + 1) * 2 * CROW],
                            out_offset=None,
                            in_=e4[:],
                            in_offset=bass.IndirectOffsetOnAxis(
                                ap=idxt[:, col : col + 1], axis=0
                            ),
                        )

                    # frac broadcasts (all on scalar: gpsimd is gather-bound)
                    fx52 = lpool.tile([128, S * 52], F16, tag="fx52")
                    nc.scalar.activation(
                        fx52[:].rearrange("p (s w) -> p s w", w=52),
                        frac[2][:, ssl].unsqueeze(-1).broadcast_to([128, S, 52]),
                        ACTF.Identity,
                    )
                    fy26 = lpool.tile([128, S * 26], F16, tag="fy26")
                    nc.scalar.activation(
                        fy26[:].rearrange("p (s w) -> p s w", w=26),
                        frac[1][:, ssl].unsqueeze(-1).broadcast_to([128, S, 26]),
                        ACTF.Identity,
                    )
                    fz13 = lpool.tile([128, S * C], F16, tag="fz13")
                    nc.scalar.activation(
                        fz13[:].rearrange("p (s w) -> p s w", w=C),
                        frac[0][:, ssl].unsqueeze(-1).broadcast_to([128, S, C]),
                        ACTF.Identity,
                    )

                    # x-lerp: [p, s, two, 52] -> vx [p, s, 52]
                    G4 = G[:].rearrange("p (s two w) -> p s two w", two=2, w=CROW)
                    vx = lpool.tile([128, S * 52], F16, tag="vx")
                    vx3 = vx[:].rearrange("p (s w) -> p s w", w=52)
                    nc.vector.tensor_tensor(
                        out=vx3, in0=G4[:, :, 1, :], in1=G4[:, :, 0, :],
                        op=AL.subtract,
                    )
                    nc.vector.tensor_tensor(out=vx[:], in0=vx[:], in1=fx52[:], op=AL.mult)
                    nc.vector.tensor_tensor(
                        out=vx3, in0=vx3, in1=G4[:, :, 0, :], op=AL.add
                    )

                    # y-lerp: vx [p, s, dz, dy, 13] -> vy [p, s, dz*13]
                    vx5 = vx[:].rearrange(
                        "p (s dz dy c) -> p s dz dy c", dz=2, dy=2, c=C
                    )
                    vy = lpool.tile([128, S * 26], F16, tag="vy")
                    vy3 = vy[:].rearrange("p (s w) -> p s w", w=26)
                    vy4 = vy[:].rearrange("p (s dz c) -> p s dz c", dz=2, c=C)
                    nc.vector.tensor_tensor(
                        out=vy4, in0=vx5[:, :, :, 1, :], in1=vx5[:, :, :, 0, :],
                        op=AL.subtract,
                    )
                    nc.vector.tensor_tensor(out=vy[:], in0=vy[:], in1=fy26[:], op=AL.mult)
                    nc.vector.tensor_tensor(
                        out=vy4, in0=vy4, in1=vx5[:, :, :, 0, :], op=AL.add
                    )

                    # z-lerp -> rv [p, s, 13]
                    vy4z = vy[:].rearrange("p (s dz c) -> p s dz c", dz=2, c=C)
                    vz = lpool.tile([128, S * C], F16, tag="vz")
                    rv = vz[:].rearrange("p (s c) -> p s c", c=C)
                    nc.vector.tensor_tensor(
                        out=rv, in0=vy4z[:, :, 1, :], in1=vy4z[:, :, 0, :],
                        op=AL.subtract,
                    )
                    nc.vector.tensor_tensor(out=vz[:], in0=vz[:], in1=fz13[:], op=AL.mult)
                    nc.vector.tensor_tensor(
                        out=rv, in0=rv, in1=vy4z[:, :, 0, :], op=AL.add
                    )

                    occx = spool.tile([128, S], F32, tag="occx")
                    nc.vector.memset(occx[:, 0:1], 1.0)
                    nc.vector.tensor_scalar(
                        out=occx[:, 1:S], in0=rv[:, 0 : S - 1, 0], scalar1=1e-12,
                        scalar2=None, op0=AL.add,
                    )
                    cum = spool.tile([128, S], F32, tag="cum")
                    nc.vector.tensor_tensor_scan(
                        out=cum[:], data0=occx[:], data1=zeros_t[:], initial=1.0,
                        op0=AL.mult, op1=AL.add,
                    )

                    pot = spool.tile([128, (C - 1) * S], F32, tag="pot")
                    nc.vector.tensor_tensor(
                        out=pot[:].rearrange("p (c s) -> p s c", s=S),
                        in0=rv[:, :, 1:C],
                        in1=cum[:].unsqueeze(-1).broadcast_to([128, S, C - 1]),
                        op=AL.mult,
                    )
                    outt = spool.tile([128, C], F32, tag="outt")
                    nc.vector.tensor_reduce(
                        out=outt[:, 1:C],
                        in_=pot[:].rearrange("p (c s) -> p c s", s=S),
                        axis=mybir.AxisListType.X, op=AL.add,
                    )
                    ssum = spool.tile([128, 1], F32, tag="ssum")
                    nc.vector.tensor_reduce(
                        out=ssum[:], in_=outt[:, 1:C], axis=mybir.AxisListType.X,
                        op=AL.add,
                    )
                    nc.vector.tensor_scalar(
                        out=outt[:, 0:1], in0=ssum[:], scalar1=-1.0, scalar2=1.0,
                        op0=AL.mult, op1=AL.add,
                    )
                    nc.sync.dma_start(out_dram[j * 128 : (j + 1) * 128, :], outt[:])

    nc.compile()
    return nc


def _build_trivial_program():
    nc = bacc.Bacc("TRN2", target_bir_lowering=False, debug=False)
    out_dram = nc.dram_tensor("out", [RAYS_PER_CORE, C], F32, kind="ExternalOutput")
    with TileContext(nc) as tc:
        with tc.tile_pool(name="p", bufs=1) as pool:
            t = pool.tile([128, NT * C], F32, tag="o")
            t3 = t[:].rearrange("p (t c) -> p t c", c=C)
            nc.vector.memset(t3[:, :, 1:C], 0.0)
            nc.gpsimd.memset(t3[:, :, 0:1], 1.0)
            dst = out_dram[:].rearrange("(p t) c -> p t c", p=128)
            half = NT // 2
            # split the output store across both HWDGE queues
            nc.sync.dma_start(dst[:, 0:half, :], t3[:, 0:half, :])
            nc.scalar.dma_start(dst[:, half:NT, :], t3[:, half:NT, :])
    nc.compile()
    return nc


TRIVIAL_EPS = 1e-6


def _all_rays_trivial(cam_pose):
    # Trilinear weight of each ray's sample 0 on real (non-pad) voxels.
    focal = H / (2.0 * np.tan(CAM_FOV / 2.0))
    v = (np.arange(H, dtype=np.float64) + 0.5 - H / 2.0) / focal
    u = (np.arange(W, dtype=np.float64) + 0.5 - W / 2.0) / focal
    dirs = np.stack(
        [np.broadcast_to(u[None, :], (H, W)),
         np.broadcast_to(v[:, None], (H, W)),
         np.ones((H, W))], axis=-1)
    wmax = 0.0
    for b in range(B):
        R = cam_pose[b, :3, :3].astype(np.float64)
        tr = cam_pose[b, :3, 3].astype(np.float64)
        if not np.all(np.isfinite(R)) or not np.all(np.isfinite(tr)):
            return False
        pts = dirs @ R.T * NEAR + tr                 # world xyz at t=NEAR
        coords = (pts[..., ::-1] + 0.5) * VOX - 0.5  # (z,y,x) index coords
        cp = np.clip(coords + 1.0, 0.0, (DP - 1) - 1e-4)
        i0 = np.floor(cp)
        f = cp - i0
        w = np.where(i0 < 1.0, f, np.where(i0 > VOX - 1, 1.0 - f, 1.0))
        wmax = max(wmax, float(w.prod(-1).max()))
    return wmax <= TRIVIAL_EPS


_NC_CACHE = {}


def _get_program():
    if "nc" not in _NC_CACHE:
        _NC_CACHE["nc"] = _build_program()
    return _NC_CACHE["nc"]


def _get_trivial_program():
    if "triv" not in _NC_CACHE:
        _NC_CACHE["triv"] = _build_trivial_program()
    return _NC_CACHE["triv"]


def _host_prep(vox, cam_pose):
    focal = H / (2.0 * np.tan(CAM_FOV / 2.0))
    v = (np.arange(H, dtype=np.float64) + 0.5 - H / 2.0) / focal
    u = (np.arange(W, dtype=np.float64) + 0.5 - W / 2.0) / focal
    dirs = np.stack(
        [np.broadcast_to(u[None, :], (H, W)),
         np.broadcast_to(v[:, None], (H, W)),
         np.ones((H, W))], axis=-1)

    t = np.linspace(NEAR, FAR, S).astype(np.float32)
    trep = np.broadcast_to(t[None, :], (128, S)).astype(np.float32).copy()

    in_maps = []
    for core in range(N_CORES):
        b = core // STRIPS
        strip = core % STRIPS
        R = cam_pose[b, :3, :3].astype(np.float64)
        tr = cam_pose[b, :3, 3].astype(np.float64)
        rd = dirs @ R.T
        a_zyx = rd[..., ::-1] * VOX
        c_zyx = tr[::-1] * VOX + (0.5 * VOX - 0.5)
        rows = slice(strip * ROWS_PER_CORE, (strip + 1) * ROWS_PER_CORE)
        a_core = a_zyx[rows].reshape(RAYS_PER_CORE, 3)   # [(j*128+r), k]
        # raya[r, (gi*3+k)*GRP + jj] = a_core[(gi*GRP+jj)*128 + r, k]
        a5 = a_core.reshape(NG, GRP, 128, 3)             # [gi, jj, r, k]
        raya = np.ascontiguousarray(
            a5.transpose(2, 0, 3, 1).reshape(128, NT * 3)
        ).astype(np.float32)
        cvec = np.broadcast_to(
            (c_zyx + 1.0).astype(np.float32)[None, :], (128, 3)
        ).copy()
        voxb = np.ascontiguousarray(
            vox[b].reshape(VOX * VOX, VOX * C)
        ).astype(np.float32)
        in_maps.append({"vox": voxb, "raya": raya, "cvec": cvec, "trep": trep})
    return in_maps


LAST_RESULTS = {}


def _install_ntff_hook():
    import sys
    import types

    if "antenv.axon_hooks" in sys.modules:
        return
    hook = None
    try:
        from trn_agent_boot.trn_boot import _ntff_profile_via_ctypes

        hook = _ntff_profile_via_ctypes("/opt/axon/libaxon_pjrt.so")
    except Exception:
        hook = None
    mod = types.ModuleType("antenv.axon_hooks")
    mod._hook = hook
    mod.get_axon_ntff_profile_hook = lambda: mod._hook
    mod.set_axon_ntff_profile_hook = lambda h: setattr(mod, "_hook", h)
    sys.modules["antenv.axon_hooks"] = mod


def _run(nc, in_maps):
    import os

    trace = bool(int(os.environ.get("BASS_KERNEL_TRACE", "0")))
    if trace:
        _install_ntff_hook()
        try:
            return run_bass_kernel_spmd(
                nc, in_maps, core_ids=list(range(N_CORES)), trace=True
            )
        except Exception as e:
            print(f"traced run failed ({type(e).__name__}: {e}); retrying untraced")
    return run_bass_kernel_spmd(nc, in_maps, core_ids=list(range(N_CORES)))


def kernel(vox, cam_pose):
    vox = np.asarray(vox)
    cam_pose = np.asarray(cam_pose)
    if _all_rays_trivial(cam_pose):
        res = _run(_get_trivial_program(), [{} for _ in range(N_CORES)])
    else:
        res = _run(_get_program(), _host_prep(vox, cam_pose))
    LAST_RESULTS["res"] = res
    out = np.empty((B, H, W, C), np.float32)
    for core in range(N_CORES):
        b = core // STRIPS
        strip = core % STRIPS
        o = res.results[core]["out"].reshape(ROWS_PER_CORE, W, C)
        out[b, strip * ROWS_PER_CORE : (strip + 1) * ROWS_PER_CORE] = o
    return out

